# revision 39
# baseline (speedup 1.0000x reference)
"""Multi-head attention (RoPE + doc-masked causal) on 8 Trainium2 cores.

Sharding: tensor-parallel over heads. H=16 heads -> 2 heads/core.
Each core computes q/k/v projections for its head group (Wq/Wk/Wv column
slices), attention for its 2 heads, and a partial output projection
(Wo row slice). Host sums the 8 partial outputs.

Layout strategy (per core):
  - qT/kT [hd=128, t] computed directly by projection matmuls (lhsT=W chunk,
    rhs=xT chunk); RoPE applied in this layout (DVE multiplies with
    partition-shifted PSUM reads, Pool adds).
  - scoresT [s, t] = k @ qT via matmul(lhsT=kT_block, rhs=qT_chunk); exp on
    ACT (scale=1/sqrt(hd) folded in); softmax denominator via ones-matmul;
    PV as outT[hd, t] = v.T @ pT; normalization by 1/l broadcast across
    partitions with a K=1 bf16 matmul; final y = outT_scaled.T @ Wo rows.
  - doc_ids are sorted -> allowed region of scoresT is block-diagonal AND
    causal. Build-time specialization: fully-masked 128x512 tiles are
    skipped entirely, fully-allowed tiles skip masking, boundary tiles get
    a host-precomputed fp8 0/1 mask multiply on the Pool engine.

Perf structure:
  - Software pipeline: iteration `it` runs projections+scores for chunk
    `it` and PV/normalize/y-projection for chunk `it-1`, so the ACT exp
    pipeline of one chunk hides under the next chunk's projection matmuls,
    and score matmuls interleave with y matmuls to ride out PSUM-bank
    reuse stalls.
  - Prologue is aggregate-HBM-bound (~11 MB before chunk-0 attention), so
    loads are split by deadline across the three DGE paths: wq pieces on
    sync HWDGE, wk pieces on scalar HWDGE, xT chunk 0 + consts on gpsimd
    SWDGE in consumption order. Chunk 0's q/k projection streams k-outer
    so the PE consumes weight/x pieces as they land.
  - xT double-buffered across chunks (prefetched one chunk ahead);
    y written back one [128, 2048] DMA per row-block.
  - ~1.5us of dummy warmup matmuls bridge the runtime preamble so the PE
    HAM clock-gate is warming before the real matmuls arrive.
  - PSUM: one 4-deep ring shared by all projection/PV/y accumulations +
    a 4-deep ring for scores/denominator/broadcast = 8 banks.
"""
import sys

sys.path.insert(0, "/opt/trn_rl_repo")

import numpy as np

import concourse.bacc as bacc
from concourse import bass_utils, mybir
from concourse.tile import TileContext

dt = mybir.dt

B, T, D, H, HD = 1, 2048, 2048, 16, 128
NCORES = 8
HPC = H // NCORES          # heads per core = 2
J = HPC * HD               # head-group width per core = 256
TCH = 512                  # t-chunk (PSUM bank = 512 fp32)
NTCH = T // TCH            # 4
KC = D // 128              # 16 contraction chunks
NTB = TCH // 128           # t-blocks per chunk = 4
SCALE = 1.0 / np.sqrt(HD)
NWARM = 28                 # dummy matmuls to warm the PE clock gate
WP = 2                     # wq/wk DMA piece size (k-chunks)
XP0 = [(0, 2), (2, 6), (6, 11), (11, 16)]  # xT chunk-0 piece boundaries


def _plan(doc):
    """Per t-chunk: list of (s_block, mask_idx|None, c0, c1); masks: [128,512]."""
    doc = np.asarray(doc).astype(np.int64)
    is_sorted = bool(np.all(doc[1:] >= doc[:-1]))
    plans, masks = [], []
    for ic in range(NTCH):
        t0 = ic * TCH
        tcols = np.arange(t0, t0 + TCH)
        if is_sorted:
            s_lo = int(np.searchsorted(doc, doc[t0], side="left"))
        else:
            s_lo = 0  # scan all blocks; allowed.any() filter keeps correctness
        blocks = []
        for b in range(s_lo // 128, (t0 + TCH - 1) // 128 + 1):
            srows = np.arange(b * 128, b * 128 + 128)
            allowed = (srows[:, None] <= tcols[None, :]) & (
                doc[srows][:, None] == doc[tcols][None, :]
            )
            if not allowed.any():
                continue
            cols = np.flatnonzero(allowed.any(axis=0))
            c0, c1 = int(cols[0]), int(cols[-1]) + 1
            if allowed[:, c0:c1].all():
                blocks.append((b, None, c0, c1))
            else:
                masks.append(allowed.astype(np.float32))
                blocks.append((b, len(masks) - 1, c0, c1))
        plans.append(blocks)
    return plans, masks


def _build(plans, n_masks):
    nc = bacc.Bacc("TRN2", target_bir_lowering=False, debug=False)
    f32, bf16, f8 = dt.float32, dt.bfloat16, dt.float8e4

    xT = nc.dram_tensor("xT", [128, KC, T], bf16, kind="ExternalInput").ap()
    wq = nc.dram_tensor("wq", [128, KC, HPC, 128], bf16, kind="ExternalInput").ap()
    wk = nc.dram_tensor("wk", [128, KC, HPC, 128], bf16, kind="ExternalInput").ap()
    wv = nc.dram_tensor("wv", [128, KC, J], bf16, kind="ExternalInput").ap()
    wo = nc.dram_tensor("wo", [128, HPC, D], bf16, kind="ExternalInput").ap()
    cosT = nc.dram_tensor("cosT", [HD, T], bf16, kind="ExternalInput").ap()
    sinT = nc.dram_tensor("sinT", [HD, T], bf16, kind="ExternalInput").ap()
    onesb_d = nc.dram_tensor("onesb", [128, 1], bf16, kind="ExternalInput").ap()
    onesr_d = nc.dram_tensor("onesr", [1, 128], bf16, kind="ExternalInput").ap()
    mk = nc.dram_tensor(
        "masks", [128, max(1, n_masks), TCH], f8, kind="ExternalInput"
    ).ap()
    y = nc.dram_tensor("y", [T, D], bf16, kind="ExternalOutput").ap()

    MULT = mybir.AluOpType.mult
    EXP = mybir.ActivationFunctionType.Exp

    with TileContext(nc) as tc:
        with (
            tc.tile_pool(name="consts", bufs=1) as consts,
            tc.tile_pool(name="xtp", bufs=2) as xtp,
            tc.tile_pool(name="rope", bufs=2) as ropep,
            tc.tile_pool(name="ptp", bufs=24) as ptp,
            tc.tile_pool(name="outp", bufs=2) as outp,
            tc.tile_pool(name="smallp", bufs=2) as smallp,
            tc.tile_pool(name="yp", bufs=2) as yp,
            tc.tile_pool(name="ps", bufs=1, space="PSUM") as ps,
        ):
            # ---- PE warmup: dummy matmuls bridge the runtime preamble /
            # first input DMAs so the HAM clock gate heads to 8/8 ----
            warm_sb = consts.tile([128, 128], bf16)
            nc.vector.memset(warm_sb, 0)
            for wi in range(NWARM):
                warm_ps = ps.tile(
                    [128, 128], f32, tag="pS", bufs=4, name=f"warm_{wi}"
                )
                nc.tensor.matmul(warm_ps, warm_sb, warm_sb, start=True, stop=True)

            # ---- input DMAs, deadline-ordered, split across the 3 DGE
            # paths. wq pieces on sync, wk pieces on scalar (landing in
            # k-order in parallel for the chunk-0 k-outer stream); xT
            # chunk 0 and the remaining consts on gpsimd in the order the
            # pipeline consumes them. ----
            wq_sb = consts.tile([128, KC, HPC, 128], bf16)
            wk_sb = consts.tile([128, KC, HPC, 128], bf16)
            for k0 in range(0, KC, WP):
                ksl = slice(k0, k0 + WP)
                nc.sync.dma_start(out=wq_sb[:, ksl], in_=wq[:, ksl])
                nc.scalar.dma_start(out=wk_sb[:, ksl], in_=wk[:, ksl])

            xt_tiles = [
                xtp.tile([128, KC, TCH], bf16, tag="xt", name=f"xt_{ic}")
                for ic in range(NTCH)
            ]
            for lo, hi in XP0:
                nc.gpsimd.dma_start(
                    out=xt_tiles[0][:, lo:hi],
                    in_=xT[:, lo:hi, 0:TCH],
                )
            cos_sb = consts.tile([HD, T], bf16)
            nc.gpsimd.dma_start(out=cos_sb, in_=cosT)
            sin_sb = consts.tile([HD, T], bf16)
            nc.gpsimd.dma_start(out=sin_sb, in_=sinT)
            wv_sb = consts.tile([128, KC, J], bf16)
            nc.gpsimd.dma_start(out=wv_sb, in_=wv)
            ones_bf = consts.tile([128, 1], bf16)
            nc.gpsimd.dma_start(out=ones_bf, in_=onesb_d)
            mk_sb = consts.tile([128, max(1, n_masks), TCH], f8)
            if n_masks:
                nc.gpsimd.dma_start(out=mk_sb, in_=mk)
            ones_row = consts.tile([1, 128], bf16)
            nc.gpsimd.dma_start(out=ones_row, in_=onesr_d)
            wo_sb = consts.tile([128, HPC, D], bf16)

            # full-kernel persistent tensors
            krope_sb = consts.tile([HD, HPC, T], bf16)
            v_sb = consts.tile([128, KC, J], bf16)

            # cross-iteration pipeline state: chunk jc's probabilities
            prev_pts = None
            qrope_tiles = {}
            outT_tiles = {}

            def emit_qk_proj(ic):
                t0 = ic * TCH
                tsl = slice(t0, t0 + TCH)
                xt_t = xt_tiles[ic]
                qrope_sb = ropep.tile(
                    [HD, HPC, TCH], bf16, tag="qr", name=f"qr_{ic}"
                )
                qrope_tiles[ic] = qrope_sb
                groups = [
                    (w_sb, dname, jb)
                    for w_sb, dname in ((wq_sb, "q"), (wk_sb, "k"))
                    for jb in range(HPC)
                ]
                if ic == 0:
                    # k-outer with all 4 accumulation groups open: streams
                    # weight/x DMA pieces as they land, PE stays contiguously
                    # busy (fragmented idle would re-throttle the HAM clock
                    # gate). Ropes afterwards, ordered by head so head-0
                    # scores unblock while head-1 ropes still run.
                    psums = {}
                    for _, dname, jb in groups:
                        psums[(dname, jb)] = ps.tile(
                            [128, TCH], f32, tag="big", bufs=4,
                            name=f"qkps_{dname}_{ic}_{jb}",
                        )
                    for k in range(KC):
                        for w_sb, dname, jb in groups:
                            nc.tensor.matmul(
                                psums[(dname, jb)],
                                w_sb[:, k, jb, :],
                                xt_t[:, k, :],
                                start=(k == 0),
                                stop=(k == KC - 1),
                            )
                    for _, dname, jb in groups:
                        _emit_rope(ic, dname, jb, psums[(dname, jb)], qrope_sb, tsl)
                else:
                    for w_sb, dname, jb in groups:
                        qk_ps = ps.tile(
                            [128, TCH], f32, tag="big", bufs=4,
                            name=f"qkps_{dname}_{ic}_{jb}",
                        )
                        for k in range(KC):
                            nc.tensor.matmul(
                                qk_ps,
                                w_sb[:, k, jb, :],
                                xt_t[:, k, :],
                                start=(k == 0),
                                stop=(k == KC - 1),
                            )
                        _emit_rope(ic, dname, jb, qk_ps, qrope_sb, tsl)

            def _emit_rope(ic, dname, jb, qk_ps, qrope_sb, tsl):
                # RoPE: out = u*cos + rot(u)*sin; rot = [-u2, u1]
                csl = cos_sb[:, tsl]
                ssl = sin_sb[:, tsl]
                t1 = ropep.tile([HD, TCH], f32, tag="t1", name=f"t1_{dname}_{ic}_{jb}")
                nc.vector.scalar_tensor_tensor(
                    out=t1, in0=qk_ps, scalar=1.0, in1=csl, op0=MULT, op1=MULT,
                )
                t2 = ropep.tile([HD, TCH], f32, tag="t2", name=f"t2_{dname}_{ic}_{jb}")
                nc.vector.scalar_tensor_tensor(
                    out=t2[0:64, :], in0=qk_ps[64:128, :], scalar=-1.0,
                    in1=ssl[0:64, :], op0=MULT, op1=MULT,
                )
                nc.vector.scalar_tensor_tensor(
                    out=t2[64:128, :], in0=qk_ps[0:64, :], scalar=1.0,
                    in1=ssl[64:128, :], op0=MULT, op1=MULT,
                )
                if dname == "q":
                    nc.gpsimd.tensor_add(qrope_sb[:, jb, :], t1, t2)
                else:
                    nc.gpsimd.tensor_add(krope_sb[:, jb, tsl], t1, t2)

            def emit_v_proj(ic):
                xt_t = xt_tiles[ic]
                for tb in range(NTB):
                    v_ps = ps.tile(
                        [128, J], f32, tag="big", bufs=4, name=f"vps_{ic}_{tb}"
                    )
                    for k in range(KC):
                        nc.tensor.matmul(
                            v_ps,
                            xt_t[:, k, tb * 128 : (tb + 1) * 128],
                            wv_sb[:, k, :],
                            start=(k == 0),
                            stop=(k == KC - 1),
                        )
                    nc.scalar.copy(v_sb[:, ic * NTB + tb, :], v_ps)

            def emit_score(ic, h, b, mi, c0, c1, pts):
                sc_ps = ps.tile(
                    [128, TCH], f32, tag="pS", bufs=4, name=f"scps_{ic}_{h}_{b}"
                )
                nc.tensor.matmul(
                    sc_ps[:, c0:c1],
                    krope_sb[:, h, b * 128 : (b + 1) * 128],
                    qrope_tiles[ic][:, h, c0:c1],
                    start=True,
                    stop=True,
                )
                pt = ptp.tile([128, TCH], bf16, tag="pt", name=f"pt_{ic}_{h}_{b}")
                nc.scalar.activation(
                    pt[:, c0:c1], sc_ps[:, c0:c1], EXP, bias=0.0, scale=SCALE
                )
                if mi is not None:
                    nc.gpsimd.tensor_tensor(
                        out=pt[:, c0:c1], in0=pt[:, c0:c1],
                        in1=mk_sb[:, mi, c0:c1], op=MULT,
                    )
                pts[h].append((b, pt, c0, c1))

            def emit_pv(jc):
                """PV + denominator + normalization for chunk jc."""
                outT_sb = outp.tile(
                    [HD, HPC, TCH], bf16, tag="outT", name=f"outT_{jc}"
                )
                outT_tiles[jc] = outT_sb
                for h in range(HPC):
                    o_ps = ps.tile(
                        [HD, TCH], f32, tag="big", bufs=4, name=f"ops_{jc}_{h}"
                    )
                    d_ps = ps.tile(
                        [1, TCH], f32, tag="pS", bufs=4, name=f"dps_{jc}_{h}"
                    )
                    hpts = prev_pts[h]
                    nblk = len(hpts)
                    for i, (b, pt, c0, c1) in enumerate(hpts):
                        nc.tensor.matmul(
                            o_ps[:, c0:c1],
                            v_sb[:, b, h * HD : (h + 1) * HD],
                            pt[:, c0:c1],
                            start=(i == 0),
                            stop=(i == nblk - 1),
                        )
                        nc.tensor.matmul(
                            d_ps[:, c0:c1],
                            ones_bf,
                            pt[:, c0:c1],
                            start=(i == 0),
                            stop=(i == nblk - 1),
                        )
                    # free the PV bank right away; broadcast l with a bf16
                    # K=1 matmul, reciprocal on the broadcast, scale from SBUF
                    o_sb = smallp.tile([HD, TCH], f32, tag="osb", name=f"osb_{jc}_{h}")
                    nc.scalar.copy(o_sb, o_ps)
                    d_r = smallp.tile([1, TCH], bf16, tag="dr", name=f"dr_{jc}_{h}")
                    nc.vector.tensor_copy(d_r, d_ps)
                    bc_ps = ps.tile(
                        [128, TCH], f32, tag="pS", bufs=4, name=f"bcps_{jc}_{h}"
                    )
                    nc.tensor.matmul(bc_ps, ones_row, d_r, start=True, stop=True)
                    rec_sb = smallp.tile(
                        [128, TCH], f32, tag="rec", name=f"rec_{jc}_{h}"
                    )
                    nc.vector.reciprocal_approx_fast(out=rec_sb, in_=bc_ps)
                    nc.vector.scalar_tensor_tensor(
                        out=outT_sb[:, h, :], in0=o_sb, scalar=1.0, in1=rec_sb,
                        op0=MULT, op1=MULT,
                    )

            def emit_y_row(jc, tb):
                t0 = jc * TCH
                trow = t0 + tb * 128
                outT_sb = outT_tiles[jc]
                y_sb = yp.tile([128, D], bf16, tag="y", name=f"y_{jc}_{tb}")
                for dc in range(D // TCH):
                    y_ps = ps.tile(
                        [128, TCH], f32, tag="big", bufs=4,
                        name=f"yps_{jc}_{tb}_{dc}",
                    )
                    for h in range(HPC):
                        nc.tensor.matmul(
                            y_ps,
                            outT_sb[:, h, tb * 128 : (tb + 1) * 128],
                            wo_sb[:, h, dc * TCH : (dc + 1) * TCH],
                            start=(h == 0),
                            stop=(h == HPC - 1),
                        )
                    if dc % 2 == 0:
                        nc.vector.tensor_copy(y_sb[:, dc * TCH : (dc + 1) * TCH], y_ps)
                    else:
                        nc.scalar.copy(y_sb[:, dc * TCH : (dc + 1) * TCH], y_ps)
                    if jc == NTCH - 1:
                        # last chunk: drain per-piece so the final DMA
                        # doesn't serialize behind all four copies
                        nc.sync.dma_start(
                            out=y[trow : trow + 128, dc * TCH : (dc + 1) * TCH],
                            in_=y_sb[:, dc * TCH : (dc + 1) * TCH],
                        )
                if jc != NTCH - 1:
                    nc.sync.dma_start(out=y[trow : trow + 128, :], in_=y_sb)

            # ---- software-pipelined main loop ----
            for it in range(NTCH + 1):
                ic, jc = it, it - 1
                if ic < NTCH and ic + 1 < NTCH:  # prefetch next chunk's xT
                    nc.scalar.dma_start(
                        out=xt_tiles[ic + 1],
                        in_=xT[:, :, (ic + 1) * TCH : (ic + 2) * TCH],
                    )
                if ic < NTCH:
                    emit_qk_proj(ic)
                if it == 0:
                    nc.gpsimd.dma_start(out=wo_sb, in_=wo)
                if jc >= 0:
                    emit_pv(jc)
                if ic < NTCH:
                    emit_v_proj(ic)
                # scores for chunk ic interleaved with y rows of chunk jc
                sitems = (
                    [(h, b, mi, c0, c1) for h in range(HPC)
                     for b, mi, c0, c1 in plans[ic]]
                    if ic < NTCH else []
                )
                pts = {h: [] for h in range(HPC)}
                per = -(-len(sitems) // NTB) if sitems else 0
                # the second-to-last chunk keeps its last two y rows as
                # ready filler for the final iteration's PV/normalize chain
                ylast = 2 if jc == NTCH - 2 else (0 if it == NTCH else NTB)
                for tb in range(NTB):
                    for h, b, mi, c0, c1 in sitems[tb * per : (tb + 1) * per]:
                        emit_score(ic, h, b, mi, c0, c1, pts)
                    if jc >= 0 and tb < ylast:
                        emit_y_row(jc, tb)
                prev_pts = pts
                if it == NTCH:
                    # deferred y rows of chunk NTCH-2 pad the gaps while the
                    # last chunk's softmax normalization chains complete
                    emit_y_row(NTCH - 2, 2)
                    emit_y_row(NTCH - 1, 0)
                    emit_y_row(NTCH - 2, 3)
                    for tb in range(1, NTB):
                        emit_y_row(NTCH - 1, tb)

    nc.compile()
    return nc


_CACHE = {}
_LAST_RESULTS = None


def _get_program(doc):
    key = doc.tobytes()
    if key not in _CACHE:
        plans, masks = _plan(doc)
        nc = _build(plans, len(masks))
        _CACHE[key] = (nc, masks)
    return _CACHE[key]


def kernel(x, Wq, Wk, Wv, Wo, sin, cos, doc_ids, **kwargs):
    import ml_dtypes

    bf = ml_dtypes.bfloat16
    f8 = ml_dtypes.float8_e4m3
    x = np.asarray(x, dtype=np.float32)
    sin = np.asarray(sin, dtype=np.float32)
    cos = np.asarray(cos, dtype=np.float32)
    doc = np.asarray(doc_ids, dtype=np.int32).reshape(-1)

    nc, masks = _get_program(doc)

    xT2 = np.ascontiguousarray(
        x.reshape(T, D).T.reshape(KC, 128, T).transpose(1, 0, 2)
    ).astype(bf)
    Wq = np.asarray(Wq, dtype=np.float32).astype(bf)
    Wk = np.asarray(Wk, dtype=np.float32).astype(bf)
    Wv = np.asarray(Wv, dtype=np.float32).astype(bf)
    Wo = np.asarray(Wo, dtype=np.float32).astype(bf)
    cosT = np.ascontiguousarray(cos.T).astype(bf)
    sinT = np.ascontiguousarray(sin.T).astype(bf)
    onesb = np.ones((128, 1), bf)
    onesr = np.ones((1, 128), bf)
    mk = (
        np.ascontiguousarray(np.stack(masks).transpose(1, 0, 2)).astype(f8)
        if masks
        else np.zeros((128, 1, TCH), f8)
    )

    in_maps = []
    for c in range(NCORES):
        jsl = slice(c * J, (c + 1) * J)
        wq_c = Wq[:, jsl].reshape(KC, 128, HPC, 128).transpose(1, 0, 2, 3)
        wk_c = Wk[:, jsl].reshape(KC, 128, HPC, 128).transpose(1, 0, 2, 3)
        wv_c = Wv[:, jsl].reshape(KC, 128, J).transpose(1, 0, 2)
        wo_c = Wo[jsl, :].reshape(HPC, 128, D).transpose(1, 0, 2)
        in_maps.append(
            {
                "xT": xT2,
                "wq": np.ascontiguousarray(wq_c),
                "wk": np.ascontiguousarray(wk_c),
                "wv": np.ascontiguousarray(wv_c),
                "wo": np.ascontiguousarray(wo_c),
                "cosT": cosT,
                "sinT": sinT,
                "onesb": onesb,
                "onesr": onesr,
                "masks": mk,
            }
        )

    res = bass_utils.run_bass_kernel_spmd(
        nc, in_maps, core_ids=list(range(NCORES)), **kwargs
    )
    global _LAST_RESULTS
    _LAST_RESULTS = res
    y = np.zeros((T, D), np.float64)
    for c in range(NCORES):
        y += res.results[c]["y"].astype(np.float64)
    return y.reshape(B, T, D).astype(np.float32)
